# revision 53
# baseline (speedup 1.0000x reference)
"""Multi-head causal attention (B=2,S=2048,D=1024,H=16) on 8 TRN2 NeuronCores.

Sharding: 8 cores = 2-way batch data-parallel x 4-way head tensor-parallel
(4 heads = 256 local dims per core). Each core computes q/k/v projections for
its head group, causal attention, and a partial out-projection (w_out
row-sharded). Host sums the 4 partials per batch element and adds b_out.

On-device layout (per core, bf16 compute, f32 accumulate):
  xT   [D, S]  = x[b].T                      (dram, bf16)
  qT/kT[256,S] = Wq_l @ x.T  (PE, K=D)       heads packed 64 rows each
  v    [S,260] = x @ WvT_l   (PE), 65-stride head interleave w/ ones column
  S.T  [k, q]  = k_h @ q_h.T (PE, K=64, two heads row-packed in PE array)
  P.T  = exp(S.T/8) (ACT, causal blocks only; diag blocks masked by DVE mul)
  outT_aug [65, q] = [v_h|1].T @ P.T  (PE, K=128) -> row 64 = softmax denom
  attnT = outT_aug[0:64] * bcast(1/denom)  (K=1 PE bcast + DVE recip/mul)
  yT_partial [D, S] = W_l @ attnT  (PE, K=256, interleaved per q-chunk)

Timing builds (hw_loop=R) are software-pipelined: each For_i body ends by
emitting the NEXT iteration's input DMAs + block-0 projections, so PE stays
fed through the final normalize chain and iteration i+1 starts hot.
"""

import sys

for _p in ("/opt/trn_rl_repo", "/root/.axon_site/_ro/trn_rl_repo"):
    if _p not in sys.path:
        sys.path.append(_p)

import numpy as np
import ml_dtypes

BF16 = ml_dtypes.bfloat16

B, S, D, H = 2, 2048, 1024, 16
HD = D // H            # 64
N_CORES = 8
TP = 4                 # head groups
HL = H // TP           # 4 heads per core
DL = HL * HD           # 256 local dims
VW = HD + 1            # 65: v columns + ones column

_BUILD_CACHE = {}


def build_nc(s=S, debug=False, n_rep=1, chain=False, probe_split=False,
             hw_loop=None, rotate=True, probe_act=False, probe_pe=False):
    """Build + finalize the Bacc graph for one core (SPMD across 8).

    hw_loop=R wraps the body in a device-side For_i loop (R iterations,
    constant instruction count) — used only for marginal timing builds.
    rotate software-pipelines multi-iteration builds: each body's tail emits
    the next iteration's input DMAs + block-0 projections. Single-shot
    (n_rep=1, no hw_loop) emission is identical either way.
    """
    assert not chain, "chain mode was removed with the rotated-head refactor"
    from concourse import bacc
    import concourse.mybir as mybir
    import concourse.tile as tile
    from contextlib import nullcontext

    bf = mybir.dt.bfloat16
    f32 = mybir.dt.float32
    Exp = mybir.ActivationFunctionType.Exp
    Ident = mybir.ActivationFunctionType.Identity

    KT = s // 128          # k tiles
    QC = s // 512          # q chunks
    MT = DL // 128         # 2 (also: head pairs)
    DKT = D // 128         # 8 contraction tiles for projections

    nc = bacc.Bacc()
    xT_d = nc.declare_dram_parameter("xT", [128, s // 512, DKT, 512], bf, isOutput=False)
    wqT_d = nc.declare_dram_parameter("wqT", [128, DKT, DL], bf, isOutput=False)
    wkT_d = nc.declare_dram_parameter("wkT", [128, DKT, DL], bf, isOutput=False)
    wvT_d = nc.declare_dram_parameter("wvT", [128, DKT, DL], bf, isOutput=False)
    wo_d = nc.declare_dram_parameter("wo", [128, MT, D], bf, isOutput=False)
    mask_d = nc.declare_dram_parameter("mask", [128, 2048], bf, isOutput=False)
    bq_d = nc.declare_dram_parameter("bq", [128, MT, 1], f32, isOutput=False)
    bk_d = nc.declare_dram_parameter("bk", [128, MT, 1], f32, isOutput=False)
    bv_d = nc.declare_dram_parameter("bv", [128, HL * VW], f32, isOutput=False)
    yT_d = nc.declare_dram_parameter("yT", [D, s], bf, isOutput=True)
    yT_i = nc.dram_tensor("yT_i", [D, s], bf) if n_rep > 1 else None
    xT_i = nc.dram_tensor("xT_i", [D, s], bf) if (chain and n_rep > 1) else None
    if debug:
        dbg_q = nc.declare_dram_parameter("dbg_q", [128, DL // 128, s], f32, isOutput=True)
        dbg_k = nc.declare_dram_parameter("dbg_k", [128, DL // 128, s], f32, isOutput=True)
        dbg_v = nc.declare_dram_parameter("dbg_v", [128, s // 128, HL * VW], f32, isOutput=True)
        dbg_a = nc.declare_dram_parameter("dbg_a", [128, DL // 128, s], f32, isOutput=True)
        dbg_pt = nc.declare_dram_parameter("dbg_pt", [128, s // 128, 1024], f32, isOutput=True)
        dbg_aug = nc.declare_dram_parameter("dbg_aug", [128, 2, 512], f32, isOutput=True)
        dbg_bc = nc.declare_dram_parameter("dbg_bc", [128, 2, 512], f32, isOutput=True)

    with tile.TileContext(nc) as tc:
        with (
            tc.tile_pool(name="const", bufs=1) as cpool,
            tc.tile_pool(name="work", bufs=1) as wpool,
            tc.tile_pool(name="pt", bufs=2) as ppool,
            tc.tile_pool(name="norm", bufs=2) as npool,
            tc.tile_pool(name="ystage", bufs=4) as ypool,
            tc.tile_pool(name="qkv_ps", bufs=2, space="PSUM") as qkv_ps,
            tc.tile_pool(name="score_ps", bufs=2, space="PSUM") as score_ps,
            tc.tile_pool(name="aug_ps", bufs=2, space="PSUM") as aug_ps,
        ):
            # ---- tiles (allocated once; bufs=1 pools reuse buffers) ----
            x_sb = cpool.tile([128, DKT, s], bf)
            wq_sb = cpool.tile([128, DKT, DL], bf)
            wk_sb = cpool.tile([128, DKT, DL], bf)
            wv_sb = cpool.tile([128, DKT, DL], bf)
            wo_sb = cpool.tile([128, MT, D], bf)
            mask_sb = cpool.tile([128, 4, 512], bf)
            bq_sb = cpool.tile([128, MT, 1], f32)
            bk_sb = cpool.tile([128, MT, 1], f32)
            bvb_sb = cpool.tile([128, HL * VW], f32)
            ones_sb = cpool.tile([128, 64], bf)
            qT_sb = wpool.tile([128, MT, s], bf)
            kT_sb = wpool.tile([128, MT, s], bf)
            v_sb = wpool.tile([128, KT, HL * VW], bf)
            attnT_sb = wpool.tile([128, MT, s], bf)

            if True:
                def head_dma_a():
                    # wk/x quarter DMAs for the first k-projection, plus the
                    # small wq/bias loads.
                    for kq in range(2):
                        nc.gpsimd.dma_start(
                            wk_sb[:, 4 * kq:4 * kq + 4, :],
                            wkT_d.ap()[:, 4 * kq:4 * kq + 4, :],
                        )
                        nc.sync.dma_start(
                            x_sb[:, 4 * kq:4 * kq + 4, 0:512],
                            xT_d[:, 0, 4 * kq:4 * kq + 4, :],
                        )
                    nc.gpsimd.dma_start(wq_sb[:], wqT_d.ap())
                    nc.gpsimd.dma_start(bk_sb[:], bk_d.ap())
                    nc.gpsimd.dma_start(bq_sb[:], bq_d.ap())

                def head_dma_b():
                    for cc in range(1, s // 512):
                        nc.sync.dma_start(
                            x_sb[:, :, cc * 512:(cc + 1) * 512], xT_d[:, cc, :, :]
                        )
                    late_dmas()

                def head_mm_k0():
                    ps0 = qkv_ps.tile([128, 512], f32, tag="qk", name="proj_ps")
                    for kt in range(DKT):
                        nc.tensor.matmul(
                            ps0[:],
                            wk_sb[:, kt, 0:128],
                            x_sb[:, kt, 0:512],
                            start=(kt == 0), stop=(kt == DKT - 1),
                        )
                    # drain on ACT: this runs in the rotated tail where ACT is
                    # idle, and keeps DVE clear for the normalize(3,1) chain
                    nc.scalar.activation(
                        kT_sb[:, 0, 0:512], ps0[:], Ident, bias=bk_sb[:, 0, :]
                    )

                def late_dmas():
                    # wv/bv first: the v projection starts ~6.4µs in, the
                    # first diagonal mask isn't needed until ~9µs
                    nc.gpsimd.dma_start(wv_sb[:], wvT_d.ap())
                    nc.gpsimd.dma_start(bvb_sb[:], bv_d.ap())
                    nc.gpsimd.dma_start(mask_sb[:], mask_d.ap().rearrange("p (t n) -> p t n", n=512))
                    nc.gpsimd.dma_start(wo_sb[:], wo_d.ap())
                nc.vector.memset(ones_sb[64:65, :], 1.0)
                for kt in range(KT):
                    ones_ap = v_sb[:, kt, :].rearrange("p (h x) -> p h x", x=VW)[:, :, HD:VW]
                    nc.vector.memset(ones_ap, 1.0)

                # ---- demand-driven: per 512-block, project k/q/v then attend.
                # Projections for block b+1 (PE) overlap exp (ACT) of block b.
                def project_block(dst, w_sb, b_sb, mt, b4):
                    ps = qkv_ps.tile([128, 512], f32, tag="qk", name="proj_ps")
                    for kt in range(DKT):
                        nc.tensor.matmul(
                            ps[:],
                            w_sb[:, kt, mt * 128:(mt + 1) * 128],
                            x_sb[:, kt, b4 * 512:(b4 + 1) * 512],
                            start=(kt == 0), stop=(kt == DKT - 1),
                        )
                    nc.vector.tensor_scalar_add(
                        dst[:, mt, b4 * 512:(b4 + 1) * 512], ps[:], b_sb[:, mt, :]
                    )

                def v_block(st_):
                    ps = qkv_ps.tile([128, DL], f32, tag="qk", name="v_ps")
                    for kt in range(DKT):
                        nc.tensor.matmul(
                            ps[:],
                            x_sb[:, kt, st_ * 128:(st_ + 1) * 128],
                            wv_sb[:, kt, :],
                            start=(kt == 0), stop=(kt == DKT - 1),
                        )
                    nc.vector.tensor_add(
                        v_sb[:, st_, :].rearrange("p (h x) -> p h x", x=VW)[:, :, 0:HD],
                        ps[:].rearrange("p (h x) -> p h x", x=HD),
                        bvb_sb[:].rearrange("p (h x) -> p h x", x=VW)[:, :, 0:HD],
                    )

                def proj_units(b4):
                    """Projection work for block b4 as independently emittable
                    thunks, used as PE filler inside the attention loop."""
                    units = [
                        lambda: project_block(kT_sb, wk_sb, bk_sb, 0, b4),
                        lambda: project_block(qT_sb, wq_sb, bq_sb, 0, b4),
                    ]
                    units += [lambda st_=st_: v_block(st_) for st_ in
                              range(4 * b4, 4 * b4 + 4)]
                    units += [
                        lambda: project_block(kT_sb, wk_sb, bk_sb, 1, b4),
                        lambda: project_block(qT_sb, wq_sb, bq_sb, 1, b4),
                    ]
                    return units

                def attn_pair(qc, p, fit=None):
                    # pair p: heads 2p (rows 0:64), 2p+1 (64:128)
                    n_kt = 4 * qc + 4
                    if True:
                        pt = ppool.tile([128, KT, 1024], bf, tag="pt", name="pt")
                        aug0 = aug_ps.tile([128, 512], f32, tag="aug", name="aug0")
                        aug1 = aug_ps.tile([128, 512], f32, tag="aug", name="aug1")
                        for kt in range(n_kt):
                            # diagonal block t: columns 0:128t are fully
                            # masked under causality -> skip them in scores/
                            # exp/P@V; only the 128-wide strip
                            # [128t:128t+128) needs the mask multiply.
                            t = kt - 4 * qc
                            off = 128 * t if t >= 0 else 0
                            qlo = qc * 512 + off
                            st = score_ps.tile([128, 2, 512], f32, tag="st", name="st")
                            if probe_split:
                                mid = (off + 512) // 2
                                for lo, hi in ((off, mid), (mid, 512)):
                                    nc.tensor.matmul(
                                        st[:, 0, lo:hi],
                                        kT_sb[0:64, p, kt * 128:(kt + 1) * 128],
                                        qT_sb[0:64, p, qc * 512 + lo:qc * 512 + hi],
                                    )
                                    nc.tensor.matmul(
                                        st[:, 1, lo:hi],
                                        kT_sb[64:128, p, kt * 128:(kt + 1) * 128],
                                        qT_sb[64:128, p, qc * 512 + lo:qc * 512 + hi],
                                    )
                            else:
                                nc.tensor.matmul(
                                    st[:, 0, off:512],
                                    kT_sb[0:64, p, kt * 128:(kt + 1) * 128],
                                    qT_sb[0:64, p, qlo:(qc + 1) * 512],
                                )
                                nc.tensor.matmul(
                                    st[:, 1, off:512],
                                    kT_sb[64:128, p, kt * 128:(kt + 1) * 128],
                                    qT_sb[64:128, p, qlo:(qc + 1) * 512],
                                )
                            # exp both heads in one call:
                            # st slot h -> pt[kt, h*512+off : (h+1)*512]
                            nc.scalar.activation(
                                pt[:, kt, :].rearrange("p (h n) -> p h n", n=512)[
                                    :, :, off:512
                                ],
                                st[:, :, off:512],
                                Exp, scale=0.125,
                            )
                            if probe_act:
                                # dead-store duplicate exp: measures the HW
                                # ACT rate vs the cost model (timing probe)
                                dead = ppool.tile([128, 2, 512], bf, tag="dead", name="dead")
                                nc.scalar.activation(
                                    dead[:, :, off:512], st[:, :, off:512],
                                    Exp, scale=0.125,
                                )
                            if t >= 0:
                                nc.gpsimd.tensor_mul(
                                    pt[:, kt, off:off + 128],
                                    pt[:, kt, off:off + 128],
                                    mask_sb[:, t, off:off + 128],
                                )
                                nc.gpsimd.tensor_mul(
                                    pt[:, kt, 512 + off:512 + off + 128],
                                    pt[:, kt, 512 + off:512 + off + 128],
                                    mask_sb[:, t, off:off + 128],
                                )
                            nc.tensor.matmul(
                                aug0[0:VW, off:512],
                                v_sb[:, kt, (2 * p) * VW:(2 * p) * VW + VW],
                                pt[:, kt, off:512],
                                start=(kt == 0), stop=(kt == n_kt - 1),
                            )
                            nc.tensor.matmul(
                                aug1[0:VW, off:512],
                                v_sb[:, kt, (2 * p + 1) * VW:(2 * p + 1) * VW + VW],
                                pt[:, kt, 512 + off:1024],
                                start=(kt == 0), stop=(kt == n_kt - 1),
                            )
                            # consume fillers aggressively so drains land
                            # inside attention, not at block boundaries
                            # (sim-swept: qc0/1=3, qc2=4, qc3=3)
                            stride = 3 if qc == QC - 1 else (4 if qc == 2 else 3)
                            if fit is not None and kt % stride == stride - 1:
                                th = next(fit, None)
                                if th is not None:
                                    th()
                        if debug and p == 0 and qc == 0:
                            with tc.tile_pool(name="dbgp", bufs=1) as dpp:
                                tpt = dpp.tile([128, s // 128, 1024], f32, tag="tpt", name="tpt")
                                nc.vector.tensor_copy(tpt[:, 0:n_kt, :], pt[:, 0:n_kt, :])
                                nc.sync.dma_start(dbg_pt.ap(), tpt[:])
                                taug = dpp.tile([128, 2, 512], f32, tag="taug", name="taug")
                                nc.vector.tensor_copy(taug[:, 0, :], aug0[:])
                                nc.vector.tensor_copy(taug[:, 1, :], aug1[:])
                                nc.sync.dma_start(dbg_aug.ap(), taug[:])
                    return aug0, aug1

                def normalize(qc, p, aug0, aug1, lo=0, hi=512):
                    if True:
                        for hh, aug in ((1, aug1), (0, aug0)):
                            # custom-DVE recip only works at base partition 0
                            # on HW and cannot read PSUM: copy the denominator
                            # row to SBUF, broadcast it to partitions 0:64 with
                            # a K=1 matmul, then reciprocal the broadcast.
                            # (Pool partition_broadcast is numerically fine on
                            # HW but ~6x slower than the cost model - avoid.)
                            row = npool.tile([128, 512], bf, tag="row", name="row")
                            # ACT is measurably faster on HW than the model
                            # says; keep DVE only in the block-3 exp window
                            if qc == QC - 1 and p == 0:
                                nc.vector.tensor_copy(row[64:65, lo:hi], aug[64:65, lo:hi])
                            else:
                                nc.scalar.copy(row[64:65, lo:hi], aug[64:65, lo:hi])
                            bcp = qkv_ps.tile([128, 512], f32, tag="qk", name="bcp")
                            nc.tensor.matmul(
                                bcp[0:64, lo:hi], ones_sb[64:65, :], row[64:65, lo:hi]
                            )
                            bc = npool.tile([128, 512], f32, tag="bc", name="bc")
                            # ACT has slack except while block 3's exps are in
                            # flight (pair-0 normalize of the last block); the
                            # pair-1 chain runs in the ACT-idle tail
                            if qc == QC - 1 and p == 0:
                                nc.vector.tensor_copy(bc[0:64, lo:hi], bcp[0:64, lo:hi])
                            else:
                                nc.scalar.copy(bc[0:64, lo:hi], bcp[0:64, lo:hi])
                            nc.vector.reciprocal_approx_fast(bc[0:64, lo:hi], bc[0:64, lo:hi])
                            if debug and p == 0 and qc == 0:
                                with tc.tile_pool(name="dbgb", bufs=1) as dpb:
                                    tbc = dpb.tile([128, 512], f32, tag="tbc" + str(hh), name="tbc")
                                    nc.vector.tensor_copy(tbc[0:64, :], bc[0:64, :])
                                    nc.sync.dma_start(dbg_bc[:, hh, :], tbc[:])
                            if hh == 0:
                                nc.vector.tensor_mul(
                                    attnT_sb[0:64, p, qc * 512 + lo:qc * 512 + hi],
                                    aug[0:64, lo:hi], bc[0:64, lo:hi],
                                )
                            else:
                                sh = npool.tile([128, 512], bf, tag="sh", name="sh")
                                nc.vector.tensor_mul(
                                    sh[0:64, lo:hi], aug[0:64, lo:hi], bc[0:64, lo:hi]
                                )
                                nc.sync.dma_start(
                                    attnT_sb[64:128, p, qc * 512 + lo:qc * 512 + hi],
                                    sh[0:64, lo:hi],
                                )

                def y_unit(qc, mt8, rep=0, lo=0, hi=512, ps=None, scalar=None):
                    if ps is None:
                        ps = qkv_ps.tile([128, 512], f32, tag="qk", name="y_ps")
                    for kt2 in range(MT):
                        nc.tensor.matmul(
                            ps[:, lo:hi],
                            wo_sb[:, kt2, mt8 * 128:(mt8 + 1) * 128],
                            attnT_sb[:, kt2, qc * 512 + lo:qc * 512 + hi],
                            start=(kt2 == 0), stop=(kt2 == MT - 1),
                        )
                    y_sb = ypool.tile([128, 512], bf, tag="y", name="y_sb")
                    # qc==2 units run while ACT is exp-saturated (block 3):
                    # keep them off ACT (GPSIMD cannot read PSUM, so Pool
                    # can't help drain). qc==3 units run in the ACT-idle
                    # tail: split them evenly so DVE isn't the lone drain.
                    if scalar is not None:
                        use_scalar = scalar
                    elif qc == 2:
                        use_scalar = False
                    else:
                        use_scalar = mt8 % 2 == 0
                    if use_scalar:
                        nc.scalar.copy(y_sb[:, lo:hi], ps[:, lo:hi])
                    else:
                        nc.vector.tensor_copy(y_sb[:, lo:hi], ps[:, lo:hi])
                    yeng = nc.sync if mt8 % 2 == 0 else nc.gpsimd
                    yeng.dma_start(
                        (yT_d if rep == 0 else yT_i)[mt8 * 128:(mt8 + 1) * 128, qc * 512 + lo:qc * 512 + hi],
                        y_sb[:, lo:hi],
                    )

                def y_units(qc, rep):
                    return [lambda m=m: y_unit(qc, m, rep) for m in range(D // 128)]

                def emit_head():
                    head_dma_a()
                    head_mm_k0()
                    head_dma_b()
                    for th in proj_units(0)[1:]:
                        th()

                # body driver: projection work for block b+1 and the
                # out-projection of block b-1 are interleaved into block b's
                # attention emission so PE always has independent filler while
                # ACT (exp) and DVE (normalize) catch up. With tail_head the
                # NEXT iteration's input DMAs + block-0 projections are
                # rotated into this body's tail (software pipelining for the
                # For_i marginal loop), so PE stays fed during the final
                # normalize chain and the next iteration starts hot.
                def dead_proj():
                    # dead-store matmul block: measures the HW PE rate vs the
                    # cost model (timing probe)
                    ps = qkv_ps.tile([128, 512], f32, tag="qk", name="dead_ps")
                    for kt in range(DKT):
                        nc.tensor.matmul(
                            ps[:], wk_sb[:, kt, 0:128], x_sb[:, kt, 0:512],
                            start=(kt == 0), stop=(kt == DKT - 1),
                        )

                def body(rep, with_head, tail_head):
                    if with_head:
                        emit_head()
                    for qc in range(QC):
                        last = qc == QC - 1
                        fill = [] if last else list(proj_units(qc + 1))
                        if probe_pe:
                            fill += [dead_proj] * 4
                        if last and tail_head:
                            fill += [head_dma_a, head_dma_b]
                        if qc > 0:
                            fill += y_units(qc - 1, rep)
                        fit = iter(fill)
                        a0 = attn_pair(qc, 0, fit)
                        # fillers between the pair-0 PV tail and its
                        # normalize: PE covers the DVE row-copy latency
                        # instead of stalling in-order at the K=1 broadcast
                        # matmul.
                        for _ in range(2):
                            th = next(fit, None)
                            if th is not None:
                                th()
                        normalize(qc, 0, *a0)
                        a1 = attn_pair(qc, 1, fit)
                        if not last:
                            # two cover thunks, then normalize, then the rest
                            # of the fillers: their DVE drains would otherwise
                            # queue ahead of the normalize row copy in DVE's
                            # in-order stream and delay the whole chain.
                            for _ in range(2):
                                th = next(fit, None)
                                if th is not None:
                                    th()
                            normalize(qc, 1, *a1)
                            for th in fit:
                                th()
                        elif tail_head:
                            head_mm_k0()
                            normalize(qc, 1, *a1)
                            for th in fit:
                                th()
                            for th in proj_units(0)[1:]:
                                th()
                            for th in y_units(qc, rep):
                                th()
                        else:
                            for th in fit:
                                th()
                            normalize(qc, 1, *a1)
                            for th in y_units(qc, rep):
                                th()

                if rotate:
                    emit_head()
                loop_ctx = (
                    tc.For_i(0, hw_loop, 1) if hw_loop is not None else nullcontext()
                )
                with loop_ctx:
                    for rep in range(n_rep):
                        if rotate:
                            body(rep, with_head=False,
                                 tail_head=hw_loop is not None or rep + 1 < n_rep)
                        else:
                            body(rep, with_head=True, tail_head=False)

        if debug:
            with tc.tile_pool(name="dbg", bufs=1) as dpool:
                for name, tsrc, dst in (("q", qT_sb, dbg_q), ("k", kT_sb, dbg_k),
                                         ("v", v_sb, dbg_v), ("a", attnT_sb, dbg_a)):
                    tmp = dpool.tile(list(tsrc.shape), f32, tag="dbg"+name, name="dbg"+name)
                    nc.vector.tensor_copy(tmp[:], tsrc[:])
                    nc.sync.dma_start(dst.ap(), tmp[:])

    nc.finalize()
    return nc


def _prep_inputs(x, w_q, b_q, w_k, b_k, w_v, b_v, w_out, s=S):
    """Per-core input shards (host-side)."""
    # causal mask tiles: mask[i, t*512+j] = 1.0 if (128*t + i) <= j else 0
    i = np.arange(128)[:, None]
    j = np.arange(512)[None, :]
    mask = np.concatenate(
        [((128 * t + i) <= j).astype(np.float32) for t in range(4)], axis=1
    ).astype(BF16)

    in_maps = []
    for c in range(N_CORES):
        b, g = divmod(c, TP)
        sl = slice(g * DL, (g + 1) * DL)
        bv_row = np.zeros((1, HL * VW), np.float32)
        for h in range(HL):
            bv_row[0, h * VW:h * VW + HD] = b_v[g * DL + h * HD: g * DL + (h + 1) * HD]
        bv_row = np.broadcast_to(bv_row, (128, HL * VW)).copy()
        swz = lambda a: np.ascontiguousarray(
            a.reshape(a.shape[0] // 128, 128, a.shape[1]).transpose(1, 0, 2))
        in_maps.append({
            "xT": np.ascontiguousarray(
                x[b, :s].T.reshape(D // 128, 128, s // 512, 512).transpose(1, 2, 0, 3)
            ).astype(BF16),
            "wqT": swz(np.ascontiguousarray(w_q[sl].T)).astype(BF16),
            "wkT": swz(np.ascontiguousarray(w_k[sl].T)).astype(BF16),
            "wvT": swz(np.ascontiguousarray(w_v[sl].T)).astype(BF16),
            "wo": swz(np.ascontiguousarray(w_out[:, sl].T)).astype(BF16),
            "mask": mask,
            "bq": np.ascontiguousarray(
                b_q[sl].reshape(DL // 128, 128, 1).transpose(1, 0, 2)).astype(np.float32),
            "bk": np.ascontiguousarray(
                b_k[sl].reshape(DL // 128, 128, 1).transpose(1, 0, 2)).astype(np.float32),
            "bv": bv_row,
        })
    return in_maps


def _assemble(results, b_out, s=S):
    out = np.zeros((B, s, D), np.float32)
    for c in range(N_CORES):
        b = c // TP
        out[b] += results[c]["yT"].T.astype(np.float32)
    out += b_out.astype(np.float32)
    return out


def kernel(x, w_q, b_q, w_k, b_k, w_v, b_v, w_out, b_out):
    from concourse.bass_utils import run_bass_kernel_spmd

    x = np.asarray(x, np.float32)
    if "nc" not in _BUILD_CACHE:
        _BUILD_CACHE["nc"] = build_nc(S)
    nc = _BUILD_CACHE["nc"]
    in_maps = _prep_inputs(
        x, np.asarray(w_q, np.float32), np.asarray(b_q, np.float32),
        np.asarray(w_k, np.float32), np.asarray(b_k, np.float32),
        np.asarray(w_v, np.float32), np.asarray(b_v, np.float32),
        np.asarray(w_out, np.float32), S,
    )
    res = run_bass_kernel_spmd(nc, in_maps, core_ids=list(range(N_CORES))).results
    return _assemble(res, np.asarray(b_out, np.float32), S)



# revision 54
# speedup vs baseline: 1.0083x; 1.0083x over previous
"""Multi-head causal attention (B=2,S=2048,D=1024,H=16) on 8 TRN2 NeuronCores.

Sharding: 8 cores = 2-way batch data-parallel x 4-way head tensor-parallel
(4 heads = 256 local dims per core). Each core computes q/k/v projections for
its head group, causal attention, and a partial out-projection (w_out
row-sharded). Host sums the 4 partials per batch element and adds b_out.

On-device layout (per core, bf16 compute, f32 accumulate):
  xT   [D, S]  = x[b].T                      (dram, bf16)
  qT/kT[256,S] = Wq_l @ x.T  (PE, K=D)       heads packed 64 rows each
  v    [S,260] = x @ WvT_l   (PE), 65-stride head interleave w/ ones column
  S.T  [k, q]  = k_h @ q_h.T (PE, K=64, two heads row-packed in PE array)
  P.T  = exp(S.T/8) (ACT, causal blocks only; diag blocks masked by DVE mul)
  outT_aug [65, q] = [v_h|1].T @ P.T  (PE, K=128) -> row 64 = softmax denom
  attnT = outT_aug[0:64] * bcast(1/denom)  (K=1 PE bcast + DVE recip/mul)
  yT_partial [D, S] = W_l @ attnT  (PE, K=256, interleaved per q-chunk)

Timing builds (hw_loop=R) are software-pipelined: each For_i body ends by
emitting the NEXT iteration's input DMAs + block-0 projections, so PE stays
fed through the final normalize chain and iteration i+1 starts hot.
"""

import sys

for _p in ("/opt/trn_rl_repo", "/root/.axon_site/_ro/trn_rl_repo"):
    if _p not in sys.path:
        sys.path.append(_p)

import numpy as np
import ml_dtypes

BF16 = ml_dtypes.bfloat16

B, S, D, H = 2, 2048, 1024, 16
HD = D // H            # 64
N_CORES = 8
TP = 4                 # head groups
HL = H // TP           # 4 heads per core
DL = HL * HD           # 256 local dims
VW = HD + 1            # 65: v columns + ones column

_BUILD_CACHE = {}


def build_nc(s=S, debug=False, n_rep=1, chain=False, probe_split=False,
             hw_loop=None, rotate=True, probe_act=False, probe_pe=False):
    """Build + finalize the Bacc graph for one core (SPMD across 8).

    hw_loop=R wraps the body in a device-side For_i loop (R iterations,
    constant instruction count) — used only for marginal timing builds.
    rotate software-pipelines multi-iteration builds: each body's tail emits
    the next iteration's input DMAs + block-0 projections. Single-shot
    (n_rep=1, no hw_loop) emission is identical either way.
    """
    assert not chain, "chain mode was removed with the rotated-head refactor"
    from concourse import bacc
    import concourse.mybir as mybir
    import concourse.tile as tile
    from contextlib import nullcontext

    bf = mybir.dt.bfloat16
    f32 = mybir.dt.float32
    Exp = mybir.ActivationFunctionType.Exp
    Ident = mybir.ActivationFunctionType.Identity

    KT = s // 128          # k tiles
    QC = s // 512          # q chunks
    MT = DL // 128         # 2 (also: head pairs)
    DKT = D // 128         # 8 contraction tiles for projections

    nc = bacc.Bacc()
    xT_d = nc.declare_dram_parameter("xT", [128, s // 512, DKT, 512], bf, isOutput=False)
    wqT_d = nc.declare_dram_parameter("wqT", [128, DKT, DL], bf, isOutput=False)
    wkT_d = nc.declare_dram_parameter("wkT", [128, DKT, DL], bf, isOutput=False)
    wvT_d = nc.declare_dram_parameter("wvT", [128, DKT, DL], bf, isOutput=False)
    wo_d = nc.declare_dram_parameter("wo", [128, MT, D], bf, isOutput=False)
    mask_d = nc.declare_dram_parameter("mask", [128, 2048], bf, isOutput=False)
    bq_d = nc.declare_dram_parameter("bq", [128, MT, 1], f32, isOutput=False)
    bk_d = nc.declare_dram_parameter("bk", [128, MT, 1], f32, isOutput=False)
    bv_d = nc.declare_dram_parameter("bv", [128, HL * VW], f32, isOutput=False)
    yT_d = nc.declare_dram_parameter("yT", [D, s], bf, isOutput=True)
    yT_i = nc.dram_tensor("yT_i", [D, s], bf) if n_rep > 1 else None
    xT_i = nc.dram_tensor("xT_i", [D, s], bf) if (chain and n_rep > 1) else None
    if debug:
        dbg_q = nc.declare_dram_parameter("dbg_q", [128, DL // 128, s], f32, isOutput=True)
        dbg_k = nc.declare_dram_parameter("dbg_k", [128, DL // 128, s], f32, isOutput=True)
        dbg_v = nc.declare_dram_parameter("dbg_v", [128, s // 128, HL * VW], f32, isOutput=True)
        dbg_a = nc.declare_dram_parameter("dbg_a", [128, DL // 128, s], f32, isOutput=True)
        dbg_pt = nc.declare_dram_parameter("dbg_pt", [128, s // 128, 1024], f32, isOutput=True)
        dbg_aug = nc.declare_dram_parameter("dbg_aug", [128, 2, 512], f32, isOutput=True)
        dbg_bc = nc.declare_dram_parameter("dbg_bc", [128, 2, 512], f32, isOutput=True)

    with tile.TileContext(nc) as tc:
        with (
            tc.tile_pool(name="const", bufs=1) as cpool,
            tc.tile_pool(name="work", bufs=1) as wpool,
            tc.tile_pool(name="pt", bufs=2) as ppool,
            tc.tile_pool(name="norm", bufs=2) as npool,
            tc.tile_pool(name="ystage", bufs=4) as ypool,
            tc.tile_pool(name="qkv_ps", bufs=2, space="PSUM") as qkv_ps,
            tc.tile_pool(name="score_ps", bufs=2, space="PSUM") as score_ps,
            tc.tile_pool(name="aug_ps", bufs=2, space="PSUM") as aug_ps,
        ):
            # ---- tiles (allocated once; bufs=1 pools reuse buffers) ----
            x_sb = cpool.tile([128, DKT, s], bf)
            wq_sb = cpool.tile([128, DKT, DL], bf)
            wk_sb = cpool.tile([128, DKT, DL], bf)
            wv_sb = cpool.tile([128, DKT, DL], bf)
            wo_sb = cpool.tile([128, MT, D], bf)
            mask_sb = cpool.tile([128, 4, 512], bf)
            bq_sb = cpool.tile([128, MT, 1], f32)
            bk_sb = cpool.tile([128, MT, 1], f32)
            bvb_sb = cpool.tile([128, HL * VW], f32)
            ones_sb = cpool.tile([128, 64], bf)
            qT_sb = wpool.tile([128, MT, s], bf)
            kT_sb = wpool.tile([128, MT, s], bf)
            v_sb = wpool.tile([128, KT, HL * VW], bf)
            attnT_sb = wpool.tile([128, MT, s], bf)

            if True:
                def head_dma_a():
                    # wk/x quarter DMAs for the first k-projection, plus the
                    # small wq/bias loads.
                    for kq in range(2):
                        nc.gpsimd.dma_start(
                            wk_sb[:, 4 * kq:4 * kq + 4, :],
                            wkT_d.ap()[:, 4 * kq:4 * kq + 4, :],
                        )
                        nc.sync.dma_start(
                            x_sb[:, 4 * kq:4 * kq + 4, 0:512],
                            xT_d[:, 0, 4 * kq:4 * kq + 4, :],
                        )
                    nc.gpsimd.dma_start(wq_sb[:], wqT_d.ap())
                    nc.gpsimd.dma_start(bk_sb[:], bk_d.ap())
                    nc.gpsimd.dma_start(bq_sb[:], bq_d.ap())

                def head_dma_b():
                    for cc in range(1, s // 512):
                        nc.sync.dma_start(
                            x_sb[:, :, cc * 512:(cc + 1) * 512], xT_d[:, cc, :, :]
                        )
                    late_dmas()

                def head_mm_k0():
                    ps0 = qkv_ps.tile([128, 512], f32, tag="qk", name="proj_ps")
                    for kt in range(DKT):
                        nc.tensor.matmul(
                            ps0[:],
                            wk_sb[:, kt, 0:128],
                            x_sb[:, kt, 0:512],
                            start=(kt == 0), stop=(kt == DKT - 1),
                        )
                    # drain on ACT: this runs in the rotated tail where ACT is
                    # idle, and keeps DVE clear for the normalize(3,1) chain
                    nc.scalar.activation(
                        kT_sb[:, 0, 0:512], ps0[:], Ident, bias=bk_sb[:, 0, :]
                    )

                def late_dmas():
                    # wv/bv first: the v projection starts ~6.4µs in, the
                    # first diagonal mask isn't needed until ~9µs
                    nc.gpsimd.dma_start(wv_sb[:], wvT_d.ap())
                    nc.gpsimd.dma_start(bvb_sb[:], bv_d.ap())
                    nc.gpsimd.dma_start(mask_sb[:], mask_d.ap().rearrange("p (t n) -> p t n", n=512))
                    nc.gpsimd.dma_start(wo_sb[:], wo_d.ap())
                nc.vector.memset(ones_sb[64:65, :], 1.0)
                for kt in range(KT):
                    ones_ap = v_sb[:, kt, :].rearrange("p (h x) -> p h x", x=VW)[:, :, HD:VW]
                    nc.vector.memset(ones_ap, 1.0)

                # ---- demand-driven: per 512-block, project k/q/v then attend.
                # Projections for block b+1 (PE) overlap exp (ACT) of block b.
                def project_block(dst, w_sb, b_sb, mt, b4):
                    ps = qkv_ps.tile([128, 512], f32, tag="qk", name="proj_ps")
                    for kt in range(DKT):
                        nc.tensor.matmul(
                            ps[:],
                            w_sb[:, kt, mt * 128:(mt + 1) * 128],
                            x_sb[:, kt, b4 * 512:(b4 + 1) * 512],
                            start=(kt == 0), stop=(kt == DKT - 1),
                        )
                    nc.vector.tensor_scalar_add(
                        dst[:, mt, b4 * 512:(b4 + 1) * 512], ps[:], b_sb[:, mt, :]
                    )

                def v_block(st_):
                    ps = qkv_ps.tile([128, DL], f32, tag="qk", name="v_ps")
                    for kt in range(DKT):
                        nc.tensor.matmul(
                            ps[:],
                            x_sb[:, kt, st_ * 128:(st_ + 1) * 128],
                            wv_sb[:, kt, :],
                            start=(kt == 0), stop=(kt == DKT - 1),
                        )
                    nc.vector.tensor_add(
                        v_sb[:, st_, :].rearrange("p (h x) -> p h x", x=VW)[:, :, 0:HD],
                        ps[:].rearrange("p (h x) -> p h x", x=HD),
                        bvb_sb[:].rearrange("p (h x) -> p h x", x=VW)[:, :, 0:HD],
                    )

                def proj_units(b4):
                    """Projection work for block b4 as independently emittable
                    thunks, used as PE filler inside the attention loop."""
                    units = [
                        lambda: project_block(kT_sb, wk_sb, bk_sb, 0, b4),
                        lambda: project_block(qT_sb, wq_sb, bq_sb, 0, b4),
                    ]
                    units += [lambda st_=st_: v_block(st_) for st_ in
                              range(4 * b4, 4 * b4 + 4)]
                    units += [
                        lambda: project_block(kT_sb, wk_sb, bk_sb, 1, b4),
                        lambda: project_block(qT_sb, wq_sb, bq_sb, 1, b4),
                    ]
                    return units

                def attn_pair(qc, p, fit=None):
                    # pair p: heads 2p (rows 0:64), 2p+1 (64:128)
                    n_kt = 4 * qc + 4
                    if True:
                        pt = ppool.tile([128, KT, 1024], bf, tag="pt", name="pt")
                        aug0 = aug_ps.tile([128, 512], f32, tag="aug", name="aug0")
                        aug1 = aug_ps.tile([128, 512], f32, tag="aug", name="aug1")
                        for kt in range(n_kt):
                            # diagonal block t: columns 0:128t are fully
                            # masked under causality -> skip them in scores/
                            # exp/P@V; only the 128-wide strip
                            # [128t:128t+128) needs the mask multiply.
                            t = kt - 4 * qc
                            off = 128 * t if t >= 0 else 0
                            qlo = qc * 512 + off
                            st = score_ps.tile([128, 2, 512], f32, tag="st", name="st")
                            if probe_split:
                                mid = (off + 512) // 2
                                for lo, hi in ((off, mid), (mid, 512)):
                                    nc.tensor.matmul(
                                        st[:, 0, lo:hi],
                                        kT_sb[0:64, p, kt * 128:(kt + 1) * 128],
                                        qT_sb[0:64, p, qc * 512 + lo:qc * 512 + hi],
                                    )
                                    nc.tensor.matmul(
                                        st[:, 1, lo:hi],
                                        kT_sb[64:128, p, kt * 128:(kt + 1) * 128],
                                        qT_sb[64:128, p, qc * 512 + lo:qc * 512 + hi],
                                    )
                            else:
                                nc.tensor.matmul(
                                    st[:, 0, off:512],
                                    kT_sb[0:64, p, kt * 128:(kt + 1) * 128],
                                    qT_sb[0:64, p, qlo:(qc + 1) * 512],
                                )
                                nc.tensor.matmul(
                                    st[:, 1, off:512],
                                    kT_sb[64:128, p, kt * 128:(kt + 1) * 128],
                                    qT_sb[64:128, p, qlo:(qc + 1) * 512],
                                )
                            # exp both heads in one call:
                            # st slot h -> pt[kt, h*512+off : (h+1)*512]
                            nc.scalar.activation(
                                pt[:, kt, :].rearrange("p (h n) -> p h n", n=512)[
                                    :, :, off:512
                                ],
                                st[:, :, off:512],
                                Exp, scale=0.125,
                            )
                            if probe_act:
                                # dead-store duplicate exp: measures the HW
                                # ACT rate vs the cost model (timing probe)
                                dead = ppool.tile([128, 2, 512], bf, tag="dead", name="dead")
                                nc.scalar.activation(
                                    dead[:, :, off:512], st[:, :, off:512],
                                    Exp, scale=0.125,
                                )
                            if t >= 0:
                                nc.gpsimd.tensor_mul(
                                    pt[:, kt, off:off + 128],
                                    pt[:, kt, off:off + 128],
                                    mask_sb[:, t, off:off + 128],
                                )
                                nc.gpsimd.tensor_mul(
                                    pt[:, kt, 512 + off:512 + off + 128],
                                    pt[:, kt, 512 + off:512 + off + 128],
                                    mask_sb[:, t, off:off + 128],
                                )
                            nc.tensor.matmul(
                                aug0[0:VW, off:512],
                                v_sb[:, kt, (2 * p) * VW:(2 * p) * VW + VW],
                                pt[:, kt, off:512],
                                start=(kt == 0), stop=(kt == n_kt - 1),
                            )
                            nc.tensor.matmul(
                                aug1[0:VW, off:512],
                                v_sb[:, kt, (2 * p + 1) * VW:(2 * p + 1) * VW + VW],
                                pt[:, kt, 512 + off:1024],
                                start=(kt == 0), stop=(kt == n_kt - 1),
                            )
                            # block-3 consumes fillers aggressively so the y
                            # drains land inside attention instead of the tail.
                            # (qc0/1=3,qc2=4 looked better in sim but lost an
                            # interleaved HW A/B by ~2us - drains inside
                            # exp-busy windows cost more on HW than modeled.)
                            stride = 3 if qc == QC - 1 else (3 if qc == 2 else 2)
                            if fit is not None and kt % stride == stride - 1:
                                th = next(fit, None)
                                if th is not None:
                                    th()
                        if debug and p == 0 and qc == 0:
                            with tc.tile_pool(name="dbgp", bufs=1) as dpp:
                                tpt = dpp.tile([128, s // 128, 1024], f32, tag="tpt", name="tpt")
                                nc.vector.tensor_copy(tpt[:, 0:n_kt, :], pt[:, 0:n_kt, :])
                                nc.sync.dma_start(dbg_pt.ap(), tpt[:])
                                taug = dpp.tile([128, 2, 512], f32, tag="taug", name="taug")
                                nc.vector.tensor_copy(taug[:, 0, :], aug0[:])
                                nc.vector.tensor_copy(taug[:, 1, :], aug1[:])
                                nc.sync.dma_start(dbg_aug.ap(), taug[:])
                    return aug0, aug1

                def normalize(qc, p, aug0, aug1, lo=0, hi=512):
                    if True:
                        for hh, aug in ((1, aug1), (0, aug0)):
                            # custom-DVE recip only works at base partition 0
                            # on HW and cannot read PSUM: copy the denominator
                            # row to SBUF, broadcast it to partitions 0:64 with
                            # a K=1 matmul, then reciprocal the broadcast.
                            # (Pool partition_broadcast is numerically fine on
                            # HW but ~6x slower than the cost model - avoid.)
                            row = npool.tile([128, 512], bf, tag="row", name="row")
                            # ACT is measurably faster on HW than the model
                            # says; keep DVE only in the block-3 exp window
                            if qc == QC - 1 and p == 0:
                                nc.vector.tensor_copy(row[64:65, lo:hi], aug[64:65, lo:hi])
                            else:
                                nc.scalar.copy(row[64:65, lo:hi], aug[64:65, lo:hi])
                            bcp = qkv_ps.tile([128, 512], f32, tag="qk", name="bcp")
                            nc.tensor.matmul(
                                bcp[0:64, lo:hi], ones_sb[64:65, :], row[64:65, lo:hi]
                            )
                            bc = npool.tile([128, 512], f32, tag="bc", name="bc")
                            # ACT has slack except while block 3's exps are in
                            # flight (pair-0 normalize of the last block); the
                            # pair-1 chain runs in the ACT-idle tail
                            if qc == QC - 1 and p == 0:
                                nc.vector.tensor_copy(bc[0:64, lo:hi], bcp[0:64, lo:hi])
                            else:
                                nc.scalar.copy(bc[0:64, lo:hi], bcp[0:64, lo:hi])
                            nc.vector.reciprocal_approx_fast(bc[0:64, lo:hi], bc[0:64, lo:hi])
                            if debug and p == 0 and qc == 0:
                                with tc.tile_pool(name="dbgb", bufs=1) as dpb:
                                    tbc = dpb.tile([128, 512], f32, tag="tbc" + str(hh), name="tbc")
                                    nc.vector.tensor_copy(tbc[0:64, :], bc[0:64, :])
                                    nc.sync.dma_start(dbg_bc[:, hh, :], tbc[:])
                            if hh == 0:
                                nc.vector.tensor_mul(
                                    attnT_sb[0:64, p, qc * 512 + lo:qc * 512 + hi],
                                    aug[0:64, lo:hi], bc[0:64, lo:hi],
                                )
                            else:
                                sh = npool.tile([128, 512], bf, tag="sh", name="sh")
                                nc.vector.tensor_mul(
                                    sh[0:64, lo:hi], aug[0:64, lo:hi], bc[0:64, lo:hi]
                                )
                                nc.sync.dma_start(
                                    attnT_sb[64:128, p, qc * 512 + lo:qc * 512 + hi],
                                    sh[0:64, lo:hi],
                                )

                def y_unit(qc, mt8, rep=0, lo=0, hi=512, ps=None, scalar=None):
                    if ps is None:
                        ps = qkv_ps.tile([128, 512], f32, tag="qk", name="y_ps")
                    for kt2 in range(MT):
                        nc.tensor.matmul(
                            ps[:, lo:hi],
                            wo_sb[:, kt2, mt8 * 128:(mt8 + 1) * 128],
                            attnT_sb[:, kt2, qc * 512 + lo:qc * 512 + hi],
                            start=(kt2 == 0), stop=(kt2 == MT - 1),
                        )
                    y_sb = ypool.tile([128, 512], bf, tag="y", name="y_sb")
                    # qc==2 units run while ACT is exp-saturated (block 3):
                    # keep them off ACT (GPSIMD cannot read PSUM, so Pool
                    # can't help drain). qc==3 units run in the ACT-idle
                    # tail: split them evenly so DVE isn't the lone drain.
                    if scalar is not None:
                        use_scalar = scalar
                    elif qc == 2:
                        use_scalar = False
                    else:
                        use_scalar = mt8 % 2 == 0
                    if use_scalar:
                        nc.scalar.copy(y_sb[:, lo:hi], ps[:, lo:hi])
                    else:
                        nc.vector.tensor_copy(y_sb[:, lo:hi], ps[:, lo:hi])
                    yeng = nc.sync if mt8 % 2 == 0 else nc.gpsimd
                    yeng.dma_start(
                        (yT_d if rep == 0 else yT_i)[mt8 * 128:(mt8 + 1) * 128, qc * 512 + lo:qc * 512 + hi],
                        y_sb[:, lo:hi],
                    )

                def y_units(qc, rep):
                    return [lambda m=m: y_unit(qc, m, rep) for m in range(D // 128)]

                def emit_head():
                    head_dma_a()
                    head_mm_k0()
                    head_dma_b()
                    for th in proj_units(0)[1:]:
                        th()

                # body driver: projection work for block b+1 and the
                # out-projection of block b-1 are interleaved into block b's
                # attention emission so PE always has independent filler while
                # ACT (exp) and DVE (normalize) catch up. With tail_head the
                # NEXT iteration's input DMAs + block-0 projections are
                # rotated into this body's tail (software pipelining for the
                # For_i marginal loop), so PE stays fed during the final
                # normalize chain and the next iteration starts hot.
                def dead_proj():
                    # dead-store matmul block: measures the HW PE rate vs the
                    # cost model (timing probe)
                    ps = qkv_ps.tile([128, 512], f32, tag="qk", name="dead_ps")
                    for kt in range(DKT):
                        nc.tensor.matmul(
                            ps[:], wk_sb[:, kt, 0:128], x_sb[:, kt, 0:512],
                            start=(kt == 0), stop=(kt == DKT - 1),
                        )

                def body(rep, with_head, tail_head):
                    if with_head:
                        emit_head()
                    for qc in range(QC):
                        last = qc == QC - 1
                        fill = [] if last else list(proj_units(qc + 1))
                        if probe_pe:
                            fill += [dead_proj] * 4
                        if last and tail_head:
                            fill += [head_dma_a, head_dma_b]
                        if qc > 0:
                            fill += y_units(qc - 1, rep)
                        fit = iter(fill)
                        a0 = attn_pair(qc, 0, fit)
                        # fillers between the pair-0 PV tail and its
                        # normalize: PE covers the DVE row-copy latency
                        # instead of stalling in-order at the K=1 broadcast
                        # matmul.
                        for _ in range(2):
                            th = next(fit, None)
                            if th is not None:
                                th()
                        normalize(qc, 0, *a0)
                        a1 = attn_pair(qc, 1, fit)
                        if not last:
                            # two cover thunks, then normalize, then the rest
                            # of the fillers: their DVE drains would otherwise
                            # queue ahead of the normalize row copy in DVE's
                            # in-order stream and delay the whole chain.
                            for _ in range(2):
                                th = next(fit, None)
                                if th is not None:
                                    th()
                            normalize(qc, 1, *a1)
                            for th in fit:
                                th()
                        elif tail_head:
                            head_mm_k0()
                            normalize(qc, 1, *a1)
                            for th in fit:
                                th()
                            for th in proj_units(0)[1:]:
                                th()
                            for th in y_units(qc, rep):
                                th()
                        else:
                            for th in fit:
                                th()
                            normalize(qc, 1, *a1)
                            for th in y_units(qc, rep):
                                th()

                if rotate:
                    emit_head()
                loop_ctx = (
                    tc.For_i(0, hw_loop, 1) if hw_loop is not None else nullcontext()
                )
                with loop_ctx:
                    for rep in range(n_rep):
                        if rotate:
                            body(rep, with_head=False,
                                 tail_head=hw_loop is not None or rep + 1 < n_rep)
                        else:
                            body(rep, with_head=True, tail_head=False)

        if debug:
            with tc.tile_pool(name="dbg", bufs=1) as dpool:
                for name, tsrc, dst in (("q", qT_sb, dbg_q), ("k", kT_sb, dbg_k),
                                         ("v", v_sb, dbg_v), ("a", attnT_sb, dbg_a)):
                    tmp = dpool.tile(list(tsrc.shape), f32, tag="dbg"+name, name="dbg"+name)
                    nc.vector.tensor_copy(tmp[:], tsrc[:])
                    nc.sync.dma_start(dst.ap(), tmp[:])

    nc.finalize()
    return nc


def _prep_inputs(x, w_q, b_q, w_k, b_k, w_v, b_v, w_out, s=S):
    """Per-core input shards (host-side)."""
    # causal mask tiles: mask[i, t*512+j] = 1.0 if (128*t + i) <= j else 0
    i = np.arange(128)[:, None]
    j = np.arange(512)[None, :]
    mask = np.concatenate(
        [((128 * t + i) <= j).astype(np.float32) for t in range(4)], axis=1
    ).astype(BF16)

    in_maps = []
    for c in range(N_CORES):
        b, g = divmod(c, TP)
        sl = slice(g * DL, (g + 1) * DL)
        bv_row = np.zeros((1, HL * VW), np.float32)
        for h in range(HL):
            bv_row[0, h * VW:h * VW + HD] = b_v[g * DL + h * HD: g * DL + (h + 1) * HD]
        bv_row = np.broadcast_to(bv_row, (128, HL * VW)).copy()
        swz = lambda a: np.ascontiguousarray(
            a.reshape(a.shape[0] // 128, 128, a.shape[1]).transpose(1, 0, 2))
        in_maps.append({
            "xT": np.ascontiguousarray(
                x[b, :s].T.reshape(D // 128, 128, s // 512, 512).transpose(1, 2, 0, 3)
            ).astype(BF16),
            "wqT": swz(np.ascontiguousarray(w_q[sl].T)).astype(BF16),
            "wkT": swz(np.ascontiguousarray(w_k[sl].T)).astype(BF16),
            "wvT": swz(np.ascontiguousarray(w_v[sl].T)).astype(BF16),
            "wo": swz(np.ascontiguousarray(w_out[:, sl].T)).astype(BF16),
            "mask": mask,
            "bq": np.ascontiguousarray(
                b_q[sl].reshape(DL // 128, 128, 1).transpose(1, 0, 2)).astype(np.float32),
            "bk": np.ascontiguousarray(
                b_k[sl].reshape(DL // 128, 128, 1).transpose(1, 0, 2)).astype(np.float32),
            "bv": bv_row,
        })
    return in_maps


def _assemble(results, b_out, s=S):
    out = np.zeros((B, s, D), np.float32)
    for c in range(N_CORES):
        b = c // TP
        out[b] += results[c]["yT"].T.astype(np.float32)
    out += b_out.astype(np.float32)
    return out


def kernel(x, w_q, b_q, w_k, b_k, w_v, b_v, w_out, b_out):
    from concourse.bass_utils import run_bass_kernel_spmd

    x = np.asarray(x, np.float32)
    if "nc" not in _BUILD_CACHE:
        _BUILD_CACHE["nc"] = build_nc(S)
    nc = _BUILD_CACHE["nc"]
    in_maps = _prep_inputs(
        x, np.asarray(w_q, np.float32), np.asarray(b_q, np.float32),
        np.asarray(w_k, np.float32), np.asarray(b_k, np.float32),
        np.asarray(w_v, np.float32), np.asarray(b_v, np.float32),
        np.asarray(w_out, np.float32), S,
    )
    res = run_bass_kernel_spmd(nc, in_maps, core_ids=list(range(N_CORES))).results
    return _assemble(res, np.asarray(b_out, np.float32), S)

